# revision 3
# baseline (speedup 1.0000x reference)
"""Trainium2 kernel for nn_Capture_Data: cap = sum(spec_data*filter, axis=(1,2))
plus Poisson/Gaussian noise synthesis.

Strategy:
  - The heavy, memory-bound part (reading 2 x 235MB and reducing over the 112
    (channel, spectral) slices) runs on 8 NeuronCores, one batch element per
    core (pure data parallel).
  - The accumulation is done with sequential fp32 adds in slice order 0..111,
    which reproduces XLA:CPU's column-reduction order bit-exactly.
  - The tiny noise-synthesis tail (jax.random.poisson/normal on the
    [8,256,256,1] result) is replicated with the exact same jax ops on the
    host CPU backend with threefry keys, matching the reference bit-for-bit.
"""

import numpy as np

P = 128             # SBUF partitions
CS = 112            # 4*28 reduced slices per batch element
NPIX = 256 * 256    # pixels per batch element
FREE = NPIX // P    # 512
G = 8               # slices loaded/multiplied per group
NG = CS // G        # 14 groups
N_CORES = 8

NL_IN = 0.5
CONS = 1e-10
POISSON_GAIN = 20.0

_TRACE = False      # set by test harness to collect an NTFF profile
LAST_EXEC_NS = None

_cached = {}


def _build_bass():
    import concourse.bacc as bacc
    import concourse.mybir as mybir
    from concourse.tile import TileContext

    nc = bacc.Bacc(None, target_bir_lowering=False)
    f32 = mybir.dt.float32
    spec = nc.dram_tensor("spec", [CS, NPIX], f32, kind="ExternalInput")
    filt = nc.dram_tensor("filt", [CS, NPIX], f32, kind="ExternalInput")
    cap = nc.dram_tensor("cap", [P, FREE], f32, kind="ExternalOutput")

    with TileContext(nc) as tc:
        with (
            tc.tile_pool(name="io", bufs=3) as io_pool,
            tc.tile_pool(name="accp", bufs=1) as acc_pool,
        ):
            acc = acc_pool.tile([P, FREE], f32)
            nc.vector.memset(acc[:], 0.0)
            for g in range(NG):
                st = io_pool.tile([P, G * FREE], f32, tag="spec")
                ft = io_pool.tile([P, G * FREE], f32, tag="filt")
                prod = io_pool.tile([P, G * FREE], f32, tag="prod")
                # slice k of this group lands at free-dim columns [k*FREE, (k+1)*FREE)
                # in the canonical pixel layout: pixel = partition*FREE + i
                src_s = spec[g * G:(g + 1) * G, :].rearrange("k (p i) -> p k i", p=P)
                src_f = filt[g * G:(g + 1) * G, :].rearrange("k (p i) -> p k i", p=P)
                nc.sync.dma_start(out=st[:].rearrange("p (k i) -> p k i", k=G), in_=src_s)
                nc.sync.dma_start(out=ft[:].rearrange("p (k i) -> p k i", k=G), in_=src_f)
                nc.vector.tensor_mul(prod[:], st[:], ft[:])
                # sequential accumulation in global slice order => matches
                # XLA:CPU reduction order bit-exactly
                for k in range(G):
                    nc.vector.tensor_add(acc[:], acc[:], prod[:, k * FREE:(k + 1) * FREE])
            nc.sync.dma_start(out=cap[:], in_=acc[:])
    nc.compile()
    return nc


def _run_device(spec_data, filt_data):
    """Run the Bass kernel on 8 cores; returns cap as float32 [8,256,256]."""
    global LAST_EXEC_NS
    from concourse.bass_utils import run_bass_kernel_spmd

    if "nc" not in _cached:
        _cached["nc"] = _build_bass()
    nc = _cached["nc"]

    in_maps = []
    for b in range(N_CORES):
        in_maps.append({
            "spec": np.ascontiguousarray(spec_data[b]).reshape(CS, NPIX),
            "filt": np.ascontiguousarray(filt_data[b]).reshape(CS, NPIX),
        })
    res = run_bass_kernel_spmd(nc, in_maps, list(range(N_CORES)), trace=_TRACE)
    LAST_EXEC_NS = res.exec_time_ns
    out = np.empty((N_CORES, 256, 256), dtype=np.float32)
    for b in range(N_CORES):
        out[b] = np.asarray(res.results[b]["cap"]).reshape(256, 256)
    return out


def _noise_synthesis(cap_np):
    """Replicates the reference's jax ops bit-exactly on the CPU backend."""
    import jax
    import jax.numpy as jnp

    cpu = jax.devices("cpu")[0]
    with jax.default_device(cpu):
        cap = jnp.asarray(cap_np)  # [8,256,256,1] float32
        poisson_t = jnp.full_like(cap, POISSON_GAIN * NL_IN)
        dark_t = jnp.full_like(cap, 1.0 * NL_IN)
        gauss_t = jnp.full_like(cap, 1.0 * NL_IN)
        peak = cap + CONS

        key = jax.random.key(1, impl="threefry2x32")
        kp, kd, kg = jax.random.split(key, 3)
        pnoisy = jax.random.poisson(kp, peak).astype(cap.dtype)
        dnoisy = jax.random.poisson(kd, dark_t).astype(cap.dtype)
        gnoisy = jax.random.normal(kg, cap.shape, dtype=cap.dtype) * gauss_t

        noisy = (pnoisy + dnoisy + gnoisy) * poisson_t / 255.0
        return (
            np.asarray(noisy),
            np.asarray(peak),
            np.asarray(dark_t),
            np.asarray(gauss_t ** 2),
        )


def kernel(spec_data, filter):
    spec_data = np.asarray(spec_data, dtype=np.float32)
    filt = np.asarray(filter, dtype=np.float32)
    cap = _run_device(spec_data, filt)[..., None]  # [8,256,256,1]
    return _noise_synthesis(cap)


# revision 4
# speedup vs baseline: 1.2004x; 1.2004x over previous
"""Trainium2 kernel for nn_Capture_Data: cap = sum(spec_data*filter, axis=(1,2))
plus Poisson/Gaussian noise synthesis.

Strategy:
  - The heavy, memory-bound part (reading 2 x 235MB and reducing over the 112
    (channel, spectral) slices) runs on 8 NeuronCores, one batch element per
    core (pure data parallel).
  - The accumulation is done with sequential fp32 adds in slice order 0..111,
    which reproduces XLA:CPU's column-reduction order bit-exactly.
  - The tiny noise-synthesis tail (jax.random.poisson/normal on the
    [8,256,256,1] result) is replicated with the exact same jax ops on the
    host CPU backend with threefry keys, matching the reference bit-for-bit.
"""

import numpy as np

P = 128             # SBUF partitions
CS = 112            # 4*28 reduced slices per batch element
NPIX = 256 * 256    # pixels per batch element
FREE = NPIX // P    # 512
G = 8               # slices loaded/multiplied per group
NG = CS // G        # 14 groups
N_CORES = 8

NL_IN = 0.5
CONS = 1e-10
POISSON_GAIN = 20.0

_TRACE = False      # set by test harness to collect an NTFF profile
LAST_EXEC_NS = None

_cached = {}


def _build_bass():
    import concourse.bacc as bacc
    import concourse.mybir as mybir
    from concourse.tile import TileContext

    nc = bacc.Bacc(None, target_bir_lowering=False)
    f32 = mybir.dt.float32
    spec = nc.dram_tensor("spec", [CS, NPIX], f32, kind="ExternalInput")
    filt = nc.dram_tensor("filt", [CS, NPIX], f32, kind="ExternalInput")
    cap = nc.dram_tensor("cap", [P, FREE], f32, kind="ExternalOutput")

    with TileContext(nc) as tc:
        with (
            tc.tile_pool(name="io_s", bufs=4) as s_pool,
            tc.tile_pool(name="io_f", bufs=4) as f_pool,
            tc.tile_pool(name="io_p", bufs=2) as p_pool,
            tc.tile_pool(name="accp", bufs=1) as acc_pool,
        ):
            acc = acc_pool.tile([P, FREE], f32)
            nc.vector.memset(acc[:], 0.0)
            for g in range(NG):
                st = s_pool.tile([P, G * FREE], f32, tag="spec")
                ft = f_pool.tile([P, G * FREE], f32, tag="filt")
                prod = p_pool.tile([P, G * FREE], f32, tag="prod")
                # slice k of this group lands at free-dim columns [k*FREE, (k+1)*FREE)
                # in the canonical pixel layout: pixel = partition*FREE + i
                src_s = spec[g * G:(g + 1) * G, :].rearrange("k (p i) -> p k i", p=P)
                src_f = filt[g * G:(g + 1) * G, :].rearrange("k (p i) -> p k i", p=P)
                # two HWDGE rings (SP + ACT) so the two streams transfer in parallel
                nc.sync.dma_start(out=st[:].rearrange("p (k i) -> p k i", k=G), in_=src_s)
                nc.scalar.dma_start(out=ft[:].rearrange("p (k i) -> p k i", k=G), in_=src_f)
                nc.vector.tensor_mul(prod[:], st[:], ft[:])
                # sequential accumulation in global slice order => matches
                # XLA:CPU reduction order bit-exactly
                for k in range(G):
                    nc.vector.tensor_add(acc[:], acc[:], prod[:, k * FREE:(k + 1) * FREE])
            nc.sync.dma_start(out=cap[:], in_=acc[:])
    nc.compile()
    return nc


def _run_device(spec_data, filt_data):
    """Run the Bass kernel on 8 cores; returns cap as float32 [8,256,256]."""
    global LAST_EXEC_NS
    from concourse.bass_utils import run_bass_kernel_spmd

    if "nc" not in _cached:
        _cached["nc"] = _build_bass()
    nc = _cached["nc"]

    in_maps = []
    for b in range(N_CORES):
        in_maps.append({
            "spec": np.ascontiguousarray(spec_data[b]).reshape(CS, NPIX),
            "filt": np.ascontiguousarray(filt_data[b]).reshape(CS, NPIX),
        })
    res = run_bass_kernel_spmd(nc, in_maps, list(range(N_CORES)), trace=_TRACE)
    LAST_EXEC_NS = res.exec_time_ns
    out = np.empty((N_CORES, 256, 256), dtype=np.float32)
    for b in range(N_CORES):
        out[b] = np.asarray(res.results[b]["cap"]).reshape(256, 256)
    return out


def _noise_synthesis(cap_np):
    """Replicates the reference's jax ops bit-exactly on the CPU backend."""
    import jax
    import jax.numpy as jnp

    cpu = jax.devices("cpu")[0]
    with jax.default_device(cpu):
        cap = jnp.asarray(cap_np)  # [8,256,256,1] float32
        poisson_t = jnp.full_like(cap, POISSON_GAIN * NL_IN)
        dark_t = jnp.full_like(cap, 1.0 * NL_IN)
        gauss_t = jnp.full_like(cap, 1.0 * NL_IN)
        peak = cap + CONS

        key = jax.random.key(1, impl="threefry2x32")
        kp, kd, kg = jax.random.split(key, 3)
        pnoisy = jax.random.poisson(kp, peak).astype(cap.dtype)
        dnoisy = jax.random.poisson(kd, dark_t).astype(cap.dtype)
        gnoisy = jax.random.normal(kg, cap.shape, dtype=cap.dtype) * gauss_t

        noisy = (pnoisy + dnoisy + gnoisy) * poisson_t / 255.0
        return (
            np.asarray(noisy),
            np.asarray(peak),
            np.asarray(dark_t),
            np.asarray(gauss_t ** 2),
        )


def kernel(spec_data, filter):
    spec_data = np.asarray(spec_data, dtype=np.float32)
    filt = np.asarray(filter, dtype=np.float32)
    cap = _run_device(spec_data, filt)[..., None]  # [8,256,256,1]
    return _noise_synthesis(cap)
